# revision 16
# baseline (speedup 1.0000x reference)
"""Trainium2 Bass kernel for nn_DecoupledAttentionWeight.

Computes the five projections q_sem/k_sem/q_geo/k_geo/v of x, applies RoPE to
the geo paths, the per-head sigmoid gate + per-path scaling to q (folded into
the projection weights host-side), and returns (q_cat, k_cat, vh) shaped
(B, H, T, 128) each.

Sharding over 8 NeuronCores: 2-way data-parallel over batch (batches {0,1} /
{2,3}) x 4-way tensor-parallel over heads (4 heads per core). Each core runs
one big [8192 x 2048] @ [2048 x 1536] matmul in bf16 (full PE speed, ~3e-3
rel err against the f32 reference) with the per-head output columns packed as
[q_sem|q_geo|k_sem|k_geo|v] so the sem||geo concat is free, then RoPE on the
geo strips via DVE with broadcast access patterns.

Structure (v4):
 - bf16 x/w/out: halves all DMA traffic vs fp32 (fp8 DoubleRow was measured
   and rejected: 1 col/cycle streaming means the precision-required 3-term
   hi/lo split costs 1.5x bf16).
 - Host-side layouts give per-partition-contiguous DMA descriptors
   (x slabs 8 KiB, w 3 KiB, cos/sin 4 KiB, out 3 KiB runs).
 - k-outer/chunk-inner matmul order: the first m-tile consumes w[k]
   incrementally as weight tiles land; stationary x-tile reused across the
   3 psum chunks.
 - Startup interleave: slab 0 split into 4 k-quarter DMAs, interleaved with
   the odd-k weight tiles on the scalar ring; even-k weights then cos/sin on
   the sync ring. First matmul starts as soon as k-quarter 0 + w[0] land.
 - One fused bf16 output DMA per m-tile; the last two m-tiles split their
   postprocess+store in half to shorten the serial tail.
"""
import math
import os
import sys

import numpy as np

for _p in ("/opt/trn_rl_repo", os.path.expanduser("~/.axon_site/_ro/trn_rl_repo")):
    if os.path.isdir(_p) and _p not in sys.path:
        sys.path.insert(0, _p)

import ml_dtypes

import concourse.bacc as bacc
import concourse.mybir as mybir
import concourse.tile as tile
from concourse.bass_utils import run_bass_kernel_spmd

# Problem config (hardcoded from the nn.Module init)
D_MODEL = 2048
N_HEADS = 16
SEM_HD = 64
GEO_HD = 64
HEAD_DIM = 128
ROPE_DIM = 64
ROPE_HALF = ROPE_DIM // 2  # 32
ROPE_BASE = 10000.0
B, T = 4, 4096

# Sharding: 2 row groups (2 batches each) x 4 head groups (4 heads each)
N_CORES = 8
RG, HG = 2, 4
ROWS_PER_CORE = (B * T) // RG          # 8192
HEADS_PER_CORE = N_HEADS // HG         # 4
BLK = SEM_HD + GEO_HD + SEM_HD + GEO_HD + HEAD_DIM  # 384 cols per head
N_CORE = HEADS_PER_CORE * BLK          # 1536
K_TILES = D_MODEL // 128               # 16
M_TILES = ROWS_PER_CORE // 128         # 64
SLAB_MT = 2                            # m_tiles per input DMA slab
SLAB_ROWS = SLAB_MT * 128              # 256
N_SLABS = M_TILES // SLAB_MT           # 32
SLAB_W = K_TILES * SLAB_ROWS           # 4096 bf16 elems per partition
CHUNK = 512                            # psum bank / matmul moving size
N_CHUNKS = N_CORE // CHUNK             # 3
COS_SLOTS = T // 128                   # 32 distinct cos/sin row-tiles

_f32 = mybir.dt.float32
_bf16 = mybir.dt.bfloat16
_bf = ml_dtypes.bfloat16


def _build_nc():
    nc = bacc.Bacc("TRN2", target_bir_lowering=False, debug=False, num_devices=1)
    xs_d = nc.dram_tensor("xs", [128, N_SLABS, SLAB_W], _bf16, kind="ExternalInput")
    w_d = nc.dram_tensor("w", [K_TILES, 128, N_CORE], _bf16, kind="ExternalInput")
    # RoPE tables, per slot 64 wide: cs = [cos|sin], sc = [sin|cos]
    cs_d = nc.dram_tensor("cs", [128, COS_SLOTS * ROPE_DIM], _f32, kind="ExternalInput")
    sc_d = nc.dram_tensor("sc", [128, COS_SLOTS * ROPE_DIM], _f32, kind="ExternalInput")
    out_d = nc.dram_tensor(
        "out", [ROWS_PER_CORE, N_CORE], _bf16, kind="ExternalOutput"
    )

    with tile.TileContext(nc) as tc:
        with (
            tc.tile_pool(name="wp", bufs=1) as wp,
            tc.tile_pool(name="xp", bufs=3) as xp,
            tc.tile_pool(name="trig", bufs=1) as trigp,
            tc.tile_pool(name="stg", bufs=3) as stgp,
            tc.tile_pool(name="tmp", bufs=2) as tmpp,
            tc.tile_pool(name="ps", bufs=2, space="PSUM") as ps,
        ):
            slab_tiles = {}

            def load_slab(s, pieces=1):
                if s not in slab_tiles:
                    t = xp.tile([128, SLAB_W], _bf16, tag="xt")
                    step = SLAB_W // pieces
                    for j in range(pieces):
                        nc.scalar.dma_start(
                            t[:, j * step:(j + 1) * step],
                            xs_d.ap()[:, s, j * step:(j + 1) * step],
                        )
                    slab_tiles[s] = t
                return slab_tiles[s]

            w_tiles = [None] * K_TILES

            def load_w(k, ring):
                wt = wp.tile([128, N_CORE], _bf16, tag=f"w{k}")
                ring.dma_start(wt[:], w_d.ap()[k])
                w_tiles[k] = wt

            # Startup interleave. Scalar ring: slab0 k-quarter, then an odd
            # weight tile, alternating -- the first m-tile's k-loop consumes
            # both in arrival order. Sync ring: even weight tiles (w[0]
            # first), then the RoPE tables (needed ~25us in, before the
            # first output DMA is enqueued behind them).
            slab0 = xp.tile([128, SLAB_W], _bf16, tag="xt")
            q = SLAB_W // 4
            nc.scalar.dma_start(slab0[:, 0:q], xs_d.ap()[:, 0, 0:q])
            load_w(1, nc.scalar)
            nc.scalar.dma_start(slab0[:, q:2 * q], xs_d.ap()[:, 0, q:2 * q])
            load_w(3, nc.scalar)
            nc.scalar.dma_start(slab0[:, 2 * q:3 * q], xs_d.ap()[:, 0, 2 * q:3 * q])
            load_w(5, nc.scalar)
            nc.scalar.dma_start(slab0[:, 3 * q:4 * q], xs_d.ap()[:, 0, 3 * q:4 * q])
            for k in (7, 9, 11, 13, 15):
                load_w(k, nc.scalar)
            slab_tiles[0] = slab0
            for k in range(0, K_TILES, 2):
                load_w(k, nc.sync)

            cs_sb = trigp.tile([128, COS_SLOTS * ROPE_DIM], _f32, tag="cs")
            nc.sync.dma_start(cs_sb[:], cs_d.ap())
            sc_sb = trigp.tile([128, COS_SLOTS * ROPE_DIM], _f32, tag="sc")
            nc.sync.dma_start(sc_sb[:], sc_d.ap())
            cs_v = cs_sb[:].rearrange("p (s c) -> p s c", s=COS_SLOTS)
            sc_v = sc_sb[:].rearrange("p (s c) -> p s c", s=COS_SLOTS)

            def postprocess(psum, mt, h0, nh, ring):
                """RoPE + copies + output DMA for heads [h0, h0+nh) of m-tile
                mt, reading psum cols h0*BLK..(h0+nh)*BLK."""
                pv = psum[:, h0 * BLK:(h0 + nh) * BLK].rearrange(
                    "p (h t c) -> p h t c", h=nh, t=3
                )
                stg = stgp.tile([128, nh * BLK], _bf16, tag=f"stg{h0}{nh}")
                sv = stg[:].rearrange("p (h t c) -> p h t c", h=nh, t=3)
                slot = mt % COS_SLOTS
                cs_bc = (
                    cs_v[:, slot, :]
                    .unsqueeze(1)
                    .unsqueeze(1)
                    .broadcast_to([128, nh, 2, ROPE_DIM])
                )
                sc_bc = (
                    sc_v[:, slot, :]
                    .unsqueeze(1)
                    .unsqueeze(1)
                    .broadcast_to([128, nh, 2, ROPE_DIM])
                )
                geo = pv[:, :, 0:2, 64:128]   # [x1 | x2]
                shp = [128, nh, 2, ROPE_DIM]
                ta = tmpp.tile(shp, _f32, tag=f"ta{h0}{nh}")
                tb = tmpp.tile(shp, _f32, tag=f"tb{h0}{nh}")
                # ta = [x1*cos | x2*sin], tb = [x1*sin | x2*cos]
                nc.vector.tensor_mul(ta[:], geo, cs_bc)
                nc.vector.tensor_mul(tb[:], geo, sc_bc)
                nc.vector.tensor_sub(
                    sv[:, :, 0:2, 64:96],
                    ta[:, :, :, 0:ROPE_HALF], ta[:, :, :, ROPE_HALF:ROPE_DIM],
                )
                nc.vector.tensor_add(
                    sv[:, :, 0:2, 96:128],
                    tb[:, :, :, ROPE_HALF:ROPE_DIM], tb[:, :, :, 0:ROPE_HALF],
                )
                # sem halves of q and k
                nc.any.tensor_copy(sv[:, :, 0:2, 0:64], pv[:, :, 0:2, 0:64])
                # v
                nc.any.tensor_copy(sv[:, :, 2, :], pv[:, :, 2, :])
                m0 = mt * 128
                ring.dma_start(
                    out_d.ap()[m0:m0 + 128, h0 * BLK:(h0 + nh) * BLK], stg[:]
                )

            def mm_k(psum, xt_v, i, k):
                for c in range(N_CHUNKS):
                    nc.tensor.matmul(
                        psum[:, c * CHUNK:(c + 1) * CHUNK],
                        xt_v[:, k, i * 128:(i + 1) * 128],
                        w_tiles[k][:, c * CHUNK:(c + 1) * CHUNK],
                        start=(k == 0),
                        stop=(k == K_TILES - 1),
                    )

            for s in range(N_SLABS):
                xt_sb = load_slab(s)
                if s + 1 < N_SLABS:
                    load_slab(s + 1)
                if s + 2 < N_SLABS:
                    load_slab(s + 2)
                xt_v = xt_sb[:].rearrange("p (k m) -> p k m", k=K_TILES)

                if s == 0:
                    # Startup: interleave both m-tiles across k so each
                    # arriving weight tile feeds 6 matmuls (~its DMA time).
                    # m-tile 0 finishes its last k-tiles first so its psum
                    # drains while m-tile 1 wraps up.
                    ps_a = ps.tile([128, N_CORE], _f32, name="psum", tag="psum")
                    ps_b = ps.tile([128, N_CORE], _f32, name="psum", tag="psum")
                    stag = 4
                    for k in range(K_TILES - stag):
                        mm_k(ps_a, xt_v, 0, k)
                        mm_k(ps_b, xt_v, 1, k)
                    for k in range(K_TILES - stag, K_TILES):
                        mm_k(ps_a, xt_v, 0, k)
                    for k in range(K_TILES - stag, K_TILES):
                        mm_k(ps_b, xt_v, 1, k)
                    postprocess(ps_a, 0, 0, HEADS_PER_CORE, nc.sync)
                    postprocess(ps_b, 1, 0, HEADS_PER_CORE, nc.scalar)
                    continue

                for i in range(SLAB_MT):
                    mt = s * SLAB_MT + i
                    psum = ps.tile([128, N_CORE], _f32, name="psum", tag="psum")
                    for k in range(K_TILES):
                        mm_k(psum, xt_v, i, k)

                    ring = nc.sync if mt % 2 == 0 else nc.scalar
                    if mt >= M_TILES - 2:
                        # tail: split postprocess+store in half so the last
                        # DMA starts ~2us earlier
                        postprocess(psum, mt, 0, 2, ring)
                        postprocess(psum, mt, 2, 2,
                                    nc.scalar if mt % 2 == 0 else nc.sync)
                    else:
                        postprocess(psum, mt, 0, HEADS_PER_CORE, ring)

    nc.compile()
    return nc


_NC_CACHE = None
LAST_RESULTS = None


def _get_nc():
    global _NC_CACHE
    if _NC_CACHE is None:
        _NC_CACHE = _build_nc()
    return _NC_CACHE


def _host_tables(pos_offset):
    """cos/sin tables (f32 numpy; matches the jax reference within ulps,
    far below the bf16 noise floor)."""
    inv_freq = (
        np.float32(ROPE_BASE)
        ** (-np.arange(0, ROPE_HALF, dtype=np.float32) * np.float32(2.0 / ROPE_DIM))
    ).astype(np.float32)
    pos = np.arange(T, dtype=np.float32) + np.float32(pos_offset)
    ang = (pos[:, None] * inv_freq[None, :]).astype(np.float32)
    cos = np.cos(ang).astype(np.float32)
    sin = np.sin(ang).astype(np.float32)
    # [T, 32] -> [p, slot, 32], row t = slot*128 + p; then concat per slot:
    # cs = [cos|sin], sc = [sin|cos] (64 wide) for the fused RoPE muls.
    cos = cos.reshape(COS_SLOTS, 128, ROPE_HALF).transpose(1, 0, 2)
    sin = sin.reshape(COS_SLOTS, 128, ROPE_HALF).transpose(1, 0, 2)
    cs = np.ascontiguousarray(
        np.concatenate([cos, sin], axis=2).reshape(128, -1)
    )
    sc = np.ascontiguousarray(
        np.concatenate([sin, cos], axis=2).reshape(128, -1)
    )
    return cs, sc


def _gate(gate_logit):
    z = np.asarray(gate_logit, dtype=np.float32)
    return (np.float32(1.0) / (np.float32(1.0) + np.exp(-z))).astype(np.float32)


def kernel(x, wq_sem, wk_sem, wq_geo, wk_geo, wv, gate_logit, pos_offset):
    x = np.asarray(x, dtype=np.float32)
    wq_sem = np.asarray(wq_sem, dtype=np.float32)
    wk_sem = np.asarray(wk_sem, dtype=np.float32)
    wq_geo = np.asarray(wq_geo, dtype=np.float32)
    wk_geo = np.asarray(wk_geo, dtype=np.float32)
    wv = np.asarray(wv, dtype=np.float32)
    pos_off = int(np.asarray(pos_offset))

    g = _gate(gate_logit)  # (16,)
    sem_scale = np.float32(1.0 / math.sqrt(float(SEM_HD)))
    geo_scale = np.float32(1.0 / math.sqrt(float(GEO_HD)))
    q_sem_col = (np.float32(2.0) * g * sem_scale).astype(np.float32)   # per head
    q_geo_col = ((np.float32(2.0) - np.float32(2.0) * g) * geo_scale).astype(
        np.float32
    )

    # Per-core weight slabs, cols per head: [qsem|qgeo|ksem|kgeo|v],
    # laid out [k, p, n] so each k-tile DMA reads 3 KiB/partition runs.
    w_cores = []
    for hg in range(HG):
        cols = []
        for hl in range(HEADS_PER_CORE):
            h = hg * HEADS_PER_CORE + hl
            cols.append(wq_sem[:, h * 64:(h + 1) * 64] * q_sem_col[h])
            cols.append(wq_geo[:, h * 64:(h + 1) * 64] * q_geo_col[h])
            cols.append(wk_sem[:, h * 64:(h + 1) * 64])
            cols.append(wk_geo[:, h * 64:(h + 1) * 64])
            cols.append(wv[:, h * 128:(h + 1) * 128])
        wc = np.concatenate(cols, axis=1).astype(_bf)       # (2048, 1536)
        w_cores.append(np.ascontiguousarray(wc.reshape(K_TILES, 128, N_CORE)))

    # x -> per-row-group slab layout [p, s, k*256+m] (8 KiB contiguous
    # per partition per slab)
    xb = x.reshape(RG, N_SLABS, SLAB_ROWS, K_TILES, 128).astype(_bf)
    xs_rg = [
        np.ascontiguousarray(xb[rg].transpose(3, 0, 2, 1).reshape(128, N_SLABS, SLAB_W))
        for rg in range(RG)
    ]

    cs, sc = _host_tables(pos_off)

    in_maps = []
    for core in range(N_CORES):
        rg, hg = core // HG, core % HG
        in_maps.append(
            {"xs": xs_rg[rg], "w": w_cores[hg], "cs": cs, "sc": sc}
        )

    nc = _get_nc()
    res = run_bass_kernel_spmd(nc, in_maps, list(range(N_CORES)))
    global LAST_RESULTS
    LAST_RESULTS = res

    q_cat = np.empty((B, N_HEADS, T, HEAD_DIM), np.float32)
    k_cat = np.empty((B, N_HEADS, T, HEAD_DIM), np.float32)
    vh = np.empty((B, N_HEADS, T, HEAD_DIM), np.float32)
    for core in range(N_CORES):
        rg, hg = core // HG, core % HG
        # (8192, 1536) bf16 -> (b_local, T, h, t3, c)
        a = np.asarray(res.results[core]["out"]).astype(np.float32)
        a = a.reshape(2, T, HEADS_PER_CORE, 3, HEAD_DIM)
        for t3_idx, dst in ((0, q_cat), (1, k_cat), (2, vh)):
            dst[
                rg * 2:(rg + 1) * 2,
                hg * HEADS_PER_CORE:(hg + 1) * HEADS_PER_CORE,
            ] = a[:, :, :, t3_idx, :].transpose(0, 2, 1, 3)
    return q_cat, k_cat, vh
